# revision 22
# baseline (speedup 1.0000x reference)
"""AttentionFlow kernel for 8 TRN2 NeuronCores (Bass/Tile).

Math (per batch, masks are all-ones by problem spec):
    wx, wy, wxy = w[:D], w[D:2D], w[2D:]
    s[i,j]  = px[i] + qy[j] + sum_d P[i,d]*wxy[d]*Q[j,d] + b
    pq_att  = softmax_j(s);  pq[i,:] = sum_j pq_att[i,j] * Q[j,:]
    qp_sim  = max_j s;       qp_att = softmax_i(qp_sim)
    qp[:]   = sum_i qp_att[i] * P[i,:]   (tiled over Lp on host)

Device formulation (per core: BC=4 batches, data parallel over B):
    qt_aug[d,j] = wxy[d]*q[j,d] + wx[d]      (host; folds px into the S matmul:
                                              sum_d qt_aug[d,j]*pT[d,i] = s - qy[j] - b)
    S^T = qt_aug^T @ pT                       [j,i] in PSUM
    e   = exp(S^T + qyb[j])                   (ACT, per-partition bias qyb = qy + b, host-made)
    u   = max_j e                             (XBAR DMA transpose + DVE free-dim reduce)
    r   = sum_j e                             (DVE free-dim reduce on the same en3)
    Y^T[d,i] = q_nat[:,d]^T @ e               (PE; copied out unnormalized in bf16)
Host post (cheap, ~0.2% of FLOPs): pq = (Y^T / r).T, qp = (u/sum u) @ P,
tiled broadcast. Softmax max-subtraction is skipped (|s| <= ~7, exp safe in
f32); ratios are mathematically identical to the reference.

Host prep: batch shards 4-per-core; bf16 casts; pT transposed layout; qt_aug,
q packed into one tensor; qy+b precomputed (f32).
"""

import numpy as np
import ml_dtypes

import concourse.bass as bass
import concourse.mybir as mybir
import concourse.tile as tile
from concourse import bacc
from concourse.bass_utils import run_bass_kernel_spmd

BF16 = mybir.dt.bfloat16
F32 = mybir.dt.float32
AF = mybir.ActivationFunctionType

B, LP, LQ, D = 32, 1024, 128, 256
NCORES = 8
BC = B // NCORES        # batches per core
NK = D // 128           # d-chunks (2)
NH = LP // 512          # 512-col halves of the i axis (2)
NI = LP // 128          # i-chunks (8)

_NC_CACHE = None


def build_kernel():
    nc = bacc.Bacc("TRN2", debug=False, target_bir_lowering=False,
                   num_devices=NCORES)

    pt_in = nc.dram_tensor("pt", [BC, NK, 128, LP], BF16,
                           kind="ExternalInput").ap()
    qpk_in = nc.dram_tensor("qpack", [BC, 128, 2 * LQ + D], BF16,
                            kind="ExternalInput").ap()
    qyb_in = nc.dram_tensor("qyb", [128, BC], F32, kind="ExternalInput").ap()
    pqt_out = nc.dram_tensor("pqt", [BC, NK, 128, LP], BF16,
                             kind="ExternalOutput").ap()
    u_out = nc.dram_tensor("u", [128, BC, NI], BF16, kind="ExternalOutput").ap()
    r_out = nc.dram_tensor("r", [128, BC, NI], F32, kind="ExternalOutput").ap()

    with tile.TileContext(nc) as tc:
        with tc.tile_pool(name="const", bufs=1) as const, \
             tc.tile_pool(name="sb", bufs=2) as sb, \
             tc.tile_pool(name="ps_st", bufs=2, space="PSUM") as ps_st, \
             tc.tile_pool(name="ps_y", bufs=2, space="PSUM") as ps_y:

            # ---- all loads issued upfront, finest-grain first so batch 0
            # can start as soon as its slices land ----
            qpk = const.tile([128, BC, 2 * LQ + D], BF16)
            pt_all = const.tile([128, BC, NK, LP], BF16)
            nc.sync.dma_start(out=qpk[:, 0],
                              in_=qpk_in[0].rearrange("p c -> p c"))
            for k in range(NK):
                nc.gpsimd.dma_start(
                    out=pt_all[:, 0, k],
                    in_=pt_in[0, k].rearrange("p i -> p i"))
            for b in range(1, BC):
                nc.sync.dma_start(out=qpk[:, b],
                                  in_=qpk_in[b].rearrange("p c -> p c"))
                nc.gpsimd.dma_start(
                    out=pt_all[:, b],
                    in_=pt_in[b].rearrange("k p i -> p k i"))
            qyb = const.tile([128, BC], F32)
            nc.sync.dma_start(out=qyb[:], in_=qyb_in[:, :])
            u_all = const.tile([128, BC, NI], BF16)
            r_all = const.tile([128, BC, NI], F32)

            def head(b):
                """S^T matmuls + exp -> eT (the j-partition layout of e)."""
                st = ps_st.tile([128, LP], F32, tag="st")
                for k in range(NK):
                    lhsT = qpk[:, b, k * LQ:(k + 1) * LQ]
                    for h in range(NH):
                        nc.tensor.matmul(
                            st[:, h * 512:(h + 1) * 512], lhsT=lhsT,
                            rhs=pt_all[:, b, k, h * 512:(h + 1) * 512],
                            start=(k == 0), stop=(k == NK - 1))
                eT = sb.tile([128, LP], BF16, tag="eT")
                for h in range(NH):
                    nc.scalar.activation(
                        eT[:, h * 512:(h + 1) * 512],
                        st[:, h * 512:(h + 1) * 512],
                        AF.Exp, bias=qyb[:, b:b + 1], scale=1.0)
                return eT

            def tail(b, eT):
                """u via transpose+reduce, r via ones matmul, Y^T, stores."""
                # u[i] = max_j e[j,i], r[i] = sum_j e[j,i]: XBAR DMA
                # transpose to natural layout, then free-dim reduces
                en_sb = sb.tile([128, NI, 128], BF16, tag="en")
                nc.sync.dma_start_transpose(en_sb[:], eT[:])
                nc.vector.reduce_max(out=u_all[:, b, :], in_=en_sb[:],
                                     axis=mybir.AxisListType.X)
                nc.vector.reduce_sum(out=r_all[:, b, :], in_=en_sb[:],
                                     axis=mybir.AxisListType.X)

                # Y^T = q_nat^T @ e, drained unnormalized as bf16
                pqt_sb = sb.tile([128, NK, LP], BF16, tag="pqt_sb")
                for k in range(NK):
                    lhsT = qpk[:, b, 2 * LQ + k * 128:2 * LQ + (k + 1) * 128]
                    yt = ps_y.tile([128, LP], F32, tag="yt")
                    for h in range(NH):
                        nc.tensor.matmul(
                            yt[:, h * 512:(h + 1) * 512], lhsT=lhsT,
                            rhs=eT[:, h * 512:(h + 1) * 512],
                            start=True, stop=True)
                    if k == 0:
                        nc.scalar.copy(pqt_sb[:, k, :], yt[:])
                    else:
                        nc.vector.tensor_copy(pqt_sb[:, k, :], yt[:])
                nc.sync.dma_start(out=pqt_out[b].rearrange("k p i -> p k i"),
                                  in_=pqt_sb[:])

            # software pipeline: S_{b+1} issues on PE before Y_b so the PE
            # queue has work while exp_b runs on ACT
            prev = None
            for b in range(BC):
                eT = head(b)
                if prev is not None:
                    tail(b - 1, prev)
                prev = eT
            tail(BC - 1, prev)

            nc.sync.dma_start(out=u_out[:, :, :], in_=u_all[:])
            nc.sync.dma_start(out=r_out[:, :, :], in_=r_all[:])

    nc.compile()
    return nc


def _get_nc():
    global _NC_CACHE
    if _NC_CACHE is None:
        _NC_CACHE = build_kernel()
    return _NC_CACHE


def _make_in_maps(paragraph, query, w, b):
    bf16 = ml_dtypes.bfloat16
    w = np.asarray(w, np.float32)
    wx, wy, wxy = w[:D], w[D:2 * D], w[2 * D:]

    p32 = np.asarray(paragraph, np.float32)
    q32 = np.asarray(query, np.float32)

    # [B, D, Lq] = wxy[d]*q[j,d] + wx[d]
    qt_aug = (q32 * wxy).transpose(0, 2, 1) + wx[None, :, None]
    qpack = np.empty((B, 128, 2 * LQ + D), bf16)
    qpack[:, :, 0:LQ] = qt_aug[:, 0:128, :]
    qpack[:, :, LQ:2 * LQ] = qt_aug[:, 128:256, :]
    qpack[:, :, 2 * LQ:] = q32
    qyb = (q32 @ wy + np.float32(b)).astype(np.float32)       # [B, Lq]
    pt = p32.transpose(0, 2, 1).astype(bf16).reshape(B, NK, 128, LP)

    in_maps = []
    for m in range(NCORES):
        sl = slice(m * BC, (m + 1) * BC)
        in_maps.append({
            "pt": pt[sl],
            "qpack": np.ascontiguousarray(qpack[sl]),
            "qyb": np.ascontiguousarray(qyb[sl].T),
        })
    return in_maps, p32


def run(paragraph, query, w, b, trace=False, **trace_kwargs):
    """Compile (cached), execute on 8 cores, return ((pq, tiled_qp), results)."""
    nc = _get_nc()
    in_maps, p32 = _make_in_maps(paragraph, query, w, b)
    res = run_bass_kernel_spmd(nc, in_maps, core_ids=list(range(NCORES)),
                               trace=trace, **trace_kwargs)
    pqt = np.concatenate(
        [np.asarray(r["pqt"], np.float32) for r in res.results], axis=0)
    r_cols = np.concatenate(
        [np.asarray(r["r"], np.float32) for r in res.results], axis=1)
    r_ = r_cols.reshape(128, B, NI).transpose(1, 2, 0).reshape(B, LP)
    pq = (pqt.reshape(B, D, LP) / r_[:, None, :]).transpose(0, 2, 1)
    # u arrives as [128, BC, NI]: u[p, b, c] = max_j e at i = c*128 + p
    u_cols = np.concatenate(
        [np.asarray(r["u"], np.float32) for r in res.results], axis=1)
    u = u_cols.reshape(128, B, NI).transpose(1, 2, 0).reshape(B, LP)
    att = u / u.sum(axis=-1, keepdims=True)
    qp = (att[:, None, :] @ p32)[:, 0, :]                     # [B, D]
    tiled_qp = np.broadcast_to(qp[:, None, :], (B, LP, D))
    return (pq, tiled_qp), res


def kernel(paragraph, query, dm, qm, w, b):
    outs, _ = run(paragraph, query, w, b, trace=False)
    return outs
